# revision 14
# baseline (speedup 1.0000x reference)
"""MFABlock Trainium2 kernel: 2-launch SPMD implementation.

d_inner=256 tensors are packed half-major: [128 partitions, 2*X free], where
half h of channel d (= h*128 + p) occupies free columns [h*X, (h+1)*X).

Launch A (6 of 8 cores): per-(branch, batch) full-L mamba scan; host
pre-reverses / pre-permutes x per branch so all cores run identical code.
Launch B (8 cores): channel attention + fuse convs; core (b, q) emits output
spatial rows [16q, 16q+16) of batch b.
"""
import sys
sys.path.insert(0, "/opt/trn_rl_repo")

import numpy as np
import ml_dtypes
import concourse.bass as bass
import concourse.mybir as mybir
import concourse.tile as tile
from concourse import bass_utils
from concourse.vector_clock import ScopedClock

F32 = mybir.dt.float32
BF16 = mybir.dt.bfloat16
AF = mybir.ActivationFunctionType
OP = mybir.AluOpType

DIM = 128
D_STATE = 16
D_CONV = 4
D_INNER = 256
DT_RANK = 8
NSLICES = 4
B_SZ, H_IMG, W_IMG = 2, 64, 64
L = H_IMG * W_IMG          # 4096
NCHUNK = 4
FD = L // NCHUNK           # 1024
CH = 512                   # pre-stage chunk
NP = DT_RANK + 2 * D_STATE  # 40

NJ = 6                     # j0 window (uniform)
WIN = 20 * 64              # out_m l-window (rows 16q-1 .. 16q+19)
GR = 18 * 66               # fuse2-in padded grid (per ic-half)
GRP = GR + 2               # +2 slack for the (+1,+1) shifted read
SLA = 24 * 66              # fuse1-in padded grid (per ic-half)
EPS = 1e-5


def _patch_tile_drain():
    """Container's walrus rejects >1 sem-wait on the SP drain at TileContext
    exit; split the global-clock waits onto standalone NOPs."""
    if getattr(tile.TileContext, "_drain_patched", False):
        return

    def _patched(self, tick_clock, wait_clock):
        nc = self.nc
        probe = nc.sync.nop(nofuse=True)
        wait_clock.add_sem_waits(
            probe.ins, ScopedClock({None: tick_clock.global_clock})
        )
        si = probe.ins.sync_info
        if si is not None and len(si.on_wait) > 1:
            waits = list(si.on_wait)
            si.on_wait = waits[:1]
            for w in waits[1:]:
                extra = nc.sync.nop(nofuse=True)
                extra.ins.sync_info = mybir.SyncInfo(on_wait=[w], on_update=[])
        nc.sync.drain()
        nc.all_engine_barrier()
        assert self.sems is not None
        popped = nc._tile_sem_poison_stack.pop()
        assert popped is self._sem_poison
        nc.clear_and_free_semaphores(list(self.sems.allocated().values()))
        nc.all_engine_barrier()

    tile.TileContext._drain_and_barrier = _patched
    tile.TileContext._drain_patched = True




_WSPLIT_CTR = [0]


def _split_excess_waits(nc, max_waits=1):
    """Walrus in this container rejects >1 sem-wait on many instruction
    structs; hoist excess waits onto same-engine NOPs placed just before."""
    for fn in nc.m.functions:
        for bb in fn.blocks:
            new_insts = []
            for inst in bb.instructions:
                si = inst.sync_info
                if si is not None and len(si.on_wait) > max_waits:
                    waits = list(si.on_wait)
                    for w in waits[:-max_waits]:
                        _WSPLIT_CTR[0] += 1
                        nop = mybir.InstNoOp(
                            name=f"I-wsplit-{_WSPLIT_CTR[0]}", ins=[], outs=[])
                        nop.engine = inst.engine
                        nop.sync_info = mybir.SyncInfo(on_wait=[w],
                                                       on_update=[])
                        new_insts.append(nop)
                        nc.register_instruction(nop, overwrite=True)
                    si.on_wait = waits[-max_waits:]
                new_insts.append(inst)
            bb.instructions = new_insts


def _layernorm(nc, pool, pps, dp, xw_t, lnw_t, lnb_t, wmean_t, width, tag):
    """LN over the 128 partitions of xw_t [128, width] -> xn tile."""
    sq = pool.tile([DIM, width], F32, tag=tag + "sq")
    nc.scalar.activation(sq[:], xw_t[:], AF.Square)
    stats = pool.tile([1, 2 * width], F32, tag=tag + "st")
    NMM = 256
    for i in range(width // NMM):
        sl = slice(i * NMM, (i + 1) * NMM)
        stp = pps.tile([1, 2 * NMM], F32, tag=tag + "stp")
        nc.tensor.matmul(stp[:, 0:NMM], wmean_t[:], xw_t[:, sl])
        nc.tensor.matmul(stp[:, NMM:2 * NMM], wmean_t[:], sq[:, sl])
        nc.scalar.copy(stats[:, i * NMM:(i + 1) * NMM], stp[:, 0:NMM])
        nc.scalar.copy(stats[:, width + i * NMM:width + (i + 1) * NMM],
                       stp[:, NMM:2 * NMM])
    musq = pool.tile([1, width], F32, tag=tag + "mq")
    nc.scalar.activation(musq[:], stats[:, 0:width], AF.Square)
    var = pool.tile([1, width], F32, tag=tag + "var")
    nc.vector.tensor_sub(var[:], stats[:, width:2 * width], musq[:])
    eps_t = pool.tile([1, 1], F32, tag=tag + "eps")
    nc.vector.memset(eps_t[:], EPS)
    lv = pool.tile([1, width], F32, tag=tag + "sd")
    nc.scalar.activation(lv[:], var[:], AF.Ln, bias=eps_t[:])
    rr = pool.tile([1, width], F32, tag=tag + "rr")
    nc.scalar.activation(rr[:], lv[:], AF.Exp, scale=-0.5)
    mr = pool.tile([1, width], F32, tag=tag + "mr")
    nc.vector.tensor_mul(mr[:], stats[:, 0:width], rr[:])
    rowd = dp.tile([2, width], F32, tag=tag + "rowd")
    nc.sync.dma_start(rowd[0:1, :], rr[:])
    nc.sync.dma_start(rowd[1:2, :], mr[:])
    R128 = pool.tile([DIM, width], F32, tag=tag + "R")
    nc.sync.dma_start(R128[:], rowd[0:1, :].partition_broadcast(DIM))
    M128 = pool.tile([DIM, width], F32, tag=tag + "M")
    nc.sync.dma_start(M128[:], rowd[1:2, :].partition_broadcast(DIM))
    t1 = pool.tile([DIM, width], F32, tag=tag + "t1")
    nc.vector.tensor_mul(t1[:], xw_t[:], R128[:])
    nc.vector.tensor_sub(t1[:], t1[:], M128[:])
    nc.vector.tensor_scalar(t1[:], t1[:], lnw_t[:], lnb_t[:], OP.mult, OP.add)
    return t1


# ---------------------------------------------------------------------------
# Launch A
# ---------------------------------------------------------------------------
def build_scan_nc():
    _patch_tile_drain()
    nc = bass.Bass("TRN2", num_devices=8, debug=False)
    xs = nc.dram_tensor("xs", [DIM, L], F32, kind="ExternalInput").ap()
    w_u_T = nc.dram_tensor("w_u_T", [DIM, D_INNER], F32, kind="ExternalInput").ap()
    ln_w = nc.dram_tensor("ln_w", [DIM, 1], F32, kind="ExternalInput").ap()
    ln_b = nc.dram_tensor("ln_b", [DIM, 1], F32, kind="ExternalInput").ap()
    w_mean = nc.dram_tensor("w_mean", [DIM, 1], F32, kind="ExternalInput").ap()
    conv_w = nc.dram_tensor("conv_w", [DIM, 2 * D_CONV], F32,
                            kind="ExternalInput").ap()
    conv_b = nc.dram_tensor("conv_b", [DIM, 2], F32, kind="ExternalInput").ap()
    xproj_T = nc.dram_tensor("xproj_T", [DIM, 2 * NP], F32,
                             kind="ExternalInput").ap()
    dtw_T = nc.dram_tensor("dtw_T", [DT_RANK, D_INNER], F32,
                           kind="ExternalInput").ap()
    dtb = nc.dram_tensor("dtb", [DIM, 2], F32, kind="ExternalInput").ap()
    A_mat = nc.dram_tensor("A_mat", [DIM, 2 * D_STATE], F32,
                           kind="ExternalInput").ap()
    Dvec = nc.dram_tensor("Dvec", [DIM, 2], F32, kind="ExternalInput").ap()
    y_out = nc.dram_tensor("y_out", [DIM, 2 * L], F32, kind="ExternalOutput").ap()

    LP = L + 3  # padded per-half width for conv input

    with tile.TileContext(nc) as tc:
        with tc.tile_pool(name="const", bufs=1) as cpool:
            lnw_t = cpool.tile([DIM, 1], F32); nc.sync.dma_start(lnw_t[:], ln_w)
            lnb_t = cpool.tile([DIM, 1], F32); nc.sync.dma_start(lnb_t[:], ln_b)
            wmean_t = cpool.tile([DIM, 1], F32)
            nc.sync.dma_start(wmean_t[:], w_mean)
            wu_t = cpool.tile([DIM, D_INNER], F32)
            nc.sync.dma_start(wu_t[:], w_u_T)
            cw_t = cpool.tile([DIM, 2 * D_CONV], F32)
            nc.sync.dma_start(cw_t[:], conv_w)
            cb_t = cpool.tile([DIM, 2], F32); nc.sync.dma_start(cb_t[:], conv_b)
            xp_t = cpool.tile([DIM, 2 * NP], F32)
            nc.sync.dma_start(xp_t[:], xproj_T)
            dtw_t = cpool.tile([DT_RANK, D_INNER], F32)
            nc.sync.dma_start(dtw_t[:], dtw_T)
            dtb_t = cpool.tile([DIM, 2], F32); nc.sync.dma_start(dtb_t[:], dtb)
            A_t = cpool.tile([DIM, 2 * D_STATE], F32)
            nc.sync.dma_start(A_t[:], A_mat)
            D_t = cpool.tile([DIM, 2], F32); nc.sync.dma_start(D_t[:], Dvec)

            with tc.tile_pool(name="persist", bufs=1) as pp:
                u_bf = pp.tile([DIM, 2 * LP], BF16, tag="ubf")
                uc_t = pp.tile([DIM, 2 * L], F32, tag="uc")
                delta_t = pp.tile([DIM, 2 * L], F32, tag="delta")
                du_t = pp.tile([DIM, 2 * L], BF16, tag="du")
                bc_t = pp.tile([NP, L], BF16, tag="bc")
                yacc_t = pp.tile([DIM, 2 * L], F32, tag="yacc")

                # ---------- pre: LN + in_proj (chunked) ----------
                for h in range(2):
                    nc.vector.memset(u_bf[:, h * LP:h * LP + 3], 0)
                with tc.tile_pool(name="pre", bufs=2) as prep, \
                     tc.tile_pool(name="lnp", bufs=1) as lnp, \
                     tc.tile_pool(name="dpre", bufs=2, space="DRAM") as dpre, \
                     tc.tile_pool(name="pps", bufs=2, space="PSUM") as pps:
                    for c in range(L // CH):
                        csl = slice(c * CH, (c + 1) * CH)
                        xc = prep.tile([DIM, CH], F32, tag="xc")
                        nc.sync.dma_start(xc[:], xs[:, csl])
                        xn = _layernorm(nc, lnp, pps, dpre, xc, lnw_t, lnb_t,
                                        wmean_t, CH, "a")
                        for i in range(CH // 512):
                            sl = slice(i * 512, (i + 1) * 512)
                            for h in range(2):
                                ups = pps.tile([128, 512], F32, tag="ups")
                                nc.tensor.matmul(
                                    ups[:], wu_t[:, h * 128:(h + 1) * 128],
                                    xn[:, sl])
                                g0 = h * LP + 3 + c * CH + i * 512
                                nc.scalar.copy(u_bf[:, g0:g0 + 512], ups[:])

                # ---------- conv + silu + projections ----------
                with tc.tile_pool(name="mid", bufs=1) as midp, \
                     tc.tile_pool(name="mps", bufs=2, space="PSUM") as mps:
                    accs = [None, None]
                    for h in range(2):
                        b0 = h * LP
                        a0 = midp.tile([DIM, L], BF16, tag=f"cva{h}")
                        nc.vector.tensor_scalar_mul(
                            a0[:], u_bf[:, b0:b0 + L], cw_t[:, h * 4:h * 4 + 1])
                        a1 = midp.tile([DIM, L], BF16, tag=f"cvb{h}")
                        nc.vector.scalar_tensor_tensor(
                            a1[:], u_bf[:, b0 + 1:b0 + 1 + L],
                            cw_t[:, h * 4 + 1:h * 4 + 2], a0[:], OP.mult, OP.add)
                        a2 = midp.tile([DIM, L], BF16, tag=f"cva{h}")
                        nc.vector.scalar_tensor_tensor(
                            a2[:], u_bf[:, b0 + 2:b0 + 2 + L],
                            cw_t[:, h * 4 + 2:h * 4 + 3], a1[:], OP.mult, OP.add)
                        a3 = midp.tile([DIM, L], BF16, tag=f"cvb{h}")
                        nc.vector.scalar_tensor_tensor(
                            a3[:], u_bf[:, b0 + 3:b0 + 3 + L],
                            cw_t[:, h * 4 + 3:h * 4 + 4], a2[:], OP.mult, OP.add)
                        nc.scalar.activation(uc_t[:, h * L:(h + 1) * L], a3[:],
                                             AF.Silu, bias=cb_t[:, h:h + 1])

                    xz = midp.tile([NP, L], F32, tag="xz")
                    for i in range(L // 512):
                        sl = slice(i * 512, (i + 1) * 512)
                        xps = mps.tile([NP, 512], F32, tag="xps")
                        for h in range(2):
                            nc.tensor.matmul(
                                xps[:], xp_t[:, h * NP:(h + 1) * NP],
                                uc_t[:, h * L + i * 512:h * L + (i + 1) * 512],
                                start=(h == 0), stop=(h == 1))
                        nc.scalar.copy(xz[:, sl], xps[:])
                    nc.vector.tensor_copy(bc_t[:], xz[:])
                    one_t = midp.tile([DIM, 1], F32, tag="one")
                    nc.vector.memset(one_t[:], 1.0)
                    for i in range(L // 512):
                        sl = slice(i * 512, (i + 1) * 512)
                        for h in range(2):
                            dps = mps.tile([128, 512], F32, tag="dps")
                            nc.tensor.matmul(
                                dps[:], dtw_t[:, h * 128:(h + 1) * 128],
                                xz[0:DT_RANK, sl])
                            edt = midp.tile([128, 512], F32, tag="edt")
                            nc.scalar.activation(edt[:], dps[:], AF.Exp,
                                                 bias=dtb_t[:, h:h + 1])
                            nc.scalar.activation(
                                delta_t[:, h * L + i * 512:
                                        h * L + (i + 1) * 512],
                                edt[:], AF.Ln, bias=one_t[:])
                    nc.vector.tensor_mul(du_t[:], delta_t[:], uc_t[:])

                # ---------- scan: n-loop ----------
                with tc.tile_pool(name="scan", bufs=2) as sp, \
                     tc.tile_pool(name="dsc", bufs=1, space="DRAM") as dsc, \
                     tc.tile_pool(name="scan3", bufs=3) as sp3:
                    bc_d = dsc.tile([NP, L], BF16, tag="bcd")
                    nc.sync.dma_start(bc_d[:], bc_t[:])
                    for n in range(D_STATE):
                        hprev = None
                        for c in range(NCHUNK):
                            sl = slice(c * FD, (c + 1) * FD)
                            Bb = sp.tile([DIM, FD], BF16, tag="Bb")
                            nc.sync.dma_start(
                                Bb[:],
                                bc_d[DT_RANK + n:DT_RANK + n + 1, sl].partition_broadcast(DIM))
                            Cb = sp.tile([DIM, FD], BF16, tag="Cb")
                            nc.sync.dma_start(
                                Cb[:],
                                bc_d[DT_RANK + D_STATE + n:DT_RANK + D_STATE + n + 1,
                                     sl].partition_broadcast(DIM))
                            dA = sp.tile([DIM, 2 * FD], BF16, tag="dA")
                            dBu = sp.tile([DIM, 2 * FD], BF16, tag="dBu")
                            hsc = sp3.tile([DIM, 2 * FD], BF16, tag="h")
                            hC = sp.tile([DIM, 2 * FD], BF16, tag="hC")
                            for h in range(2):
                                hf = slice(h * FD, (h + 1) * FD)
                                dsl = slice(h * L + c * FD, h * L + (c + 1) * FD)
                                nc.scalar.activation(
                                    dA[:, hf], delta_t[:, dsl], AF.Exp,
                                    scale=A_t[:, h * D_STATE + n:
                                              h * D_STATE + n + 1])
                                nc.gpsimd.tensor_mul(dBu[:, hf], du_t[:, dsl],
                                                     Bb[:])
                                init = (0.0 if c == 0 else
                                        hprev[:, h * FD + FD - 1:h * FD + FD])
                                nc.vector.tensor_tensor_scan(
                                    hsc[:, hf], dA[:, hf], dBu[:, hf], init,
                                    OP.mult, OP.add)
                                nc.vector.tensor_mul(hC[:, hf], hsc[:, hf],
                                                     Cb[:])
                                ysl = slice(h * L + c * FD,
                                            h * L + (c + 1) * FD)
                                if n == 0:
                                    nc.gpsimd.tensor_copy(yacc_t[:, ysl],
                                                          hC[:, hf])
                                else:
                                    nc.gpsimd.tensor_add(yacc_t[:, ysl],
                                                         yacc_t[:, ysl],
                                                         hC[:, hf])
                            hprev = hsc

                with tc.tile_pool(name="fin", bufs=1) as fp:
                    yfin = fp.tile([DIM, 2 * L], F32, tag="yfin")
                    for h in range(2):
                        hf = slice(h * L, (h + 1) * L)
                        nc.vector.scalar_tensor_tensor(
                            yfin[:, hf], uc_t[:, hf], D_t[:, h:h + 1],
                            yacc_t[:, hf], OP.mult, OP.add)
                    nc.sync.dma_start(y_out, yfin[:])
    _split_excess_waits(nc)
    return nc


# ---------------------------------------------------------------------------
# Launch B
# ---------------------------------------------------------------------------
def build_post_nc():
    _patch_tile_drain()
    nc = bass.Bass("TRN2", num_devices=8, debug=False)
    y_f = nc.dram_tensor("y_f", [DIM, 2 * L], F32, kind="ExternalInput").ap()
    y_b = nc.dram_tensor("y_b", [DIM, 2 * L], F32, kind="ExternalInput").ap()
    y_s_sl = nc.dram_tensor("y_s_sl", [DIM, 2 * NJ * 256], F32,
                            kind="ExternalInput").ap()
    y_f_w = nc.dram_tensor("y_f_w", [DIM, 2 * WIN], F32,
                           kind="ExternalInput").ap()
    y_b_w = nc.dram_tensor("y_b_w", [DIM, 2 * WIN], F32,
                           kind="ExternalInput").ap()
    y_s_w = nc.dram_tensor("y_s_w", [DIM, 2 * WIN], F32,
                           kind="ExternalInput").ap()
    x_slab = nc.dram_tensor("x_slab", [DIM, WIN], F32, kind="ExternalInput").ap()
    x_res = nc.dram_tensor("x_res", [DIM, 1024], F32, kind="ExternalInput").ap()
    w_z_T = nc.dram_tensor("w_z_T", [DIM, D_INNER], F32, kind="ExternalInput").ap()
    ln_w = nc.dram_tensor("ln_w", [DIM, 1], F32, kind="ExternalInput").ap()
    ln_b = nc.dram_tensor("ln_b", [DIM, 1], F32, kind="ExternalInput").ap()
    w_mean = nc.dram_tensor("w_mean", [DIM, 1], F32, kind="ExternalInput").ap()
    outp_T = nc.dram_tensor("outp_T", [DIM, 2 * DIM], F32,
                            kind="ExternalInput").ap()
    f1w = nc.dram_tensor("f1w", [DIM, 2 * 9 * DIM], BF16,
                         kind="ExternalInput").ap()
    f1b = nc.dram_tensor("f1b", [DIM, 1], F32, kind="ExternalInput").ap()
    f2w = nc.dram_tensor("f2w", [DIM, 2 * 9 * DIM], BF16,
                         kind="ExternalInput").ap()
    f2b = nc.dram_tensor("f2b", [DIM, 1], F32, kind="ExternalInput").ap()
    ident = nc.dram_tensor("ident", [128, 128], F32, kind="ExternalInput").ap()
    mask = nc.dram_tensor("mask", [DIM, GR], F32, kind="ExternalInput").ap()
    o_out = nc.dram_tensor("o_out", [DIM, 1024], F32, kind="ExternalOutput").ap()

    with tile.TileContext(nc) as tc:
        with tc.tile_pool(name="const", bufs=1) as cp:
            id_t = cp.tile([128, 128], F32); nc.sync.dma_start(id_t[:], ident)
            lnw_t = cp.tile([DIM, 1], F32); nc.sync.dma_start(lnw_t[:], ln_w)
            lnb_t = cp.tile([DIM, 1], F32); nc.sync.dma_start(lnb_t[:], ln_b)
            wmean_t = cp.tile([DIM, 1], F32); nc.sync.dma_start(wmean_t[:], w_mean)
            wz_t = cp.tile([DIM, D_INNER], F32); nc.sync.dma_start(wz_t[:], w_z_T)
            op_t = cp.tile([DIM, 2 * DIM], F32); nc.sync.dma_start(op_t[:], outp_T)
            f1w_t = cp.tile([DIM, 2 * 9 * DIM], BF16)
            nc.sync.dma_start(f1w_t[:], f1w)
            f1b_t = cp.tile([DIM, 1], F32); nc.sync.dma_start(f1b_t[:], f1b)
            f2w_t = cp.tile([DIM, 2 * 9 * DIM], BF16)
            nc.sync.dma_start(f2w_t[:], f2w)
            f2b_t = cp.tile([DIM, 1], F32); nc.sync.dma_start(f2b_t[:], f2b)
            mask_t = cp.tile([DIM, GR], F32); nc.sync.dma_start(mask_t[:], mask)

            with tc.tile_pool(name="big", bufs=1) as bp:
                yfT = bp.tile([128, 32 * 256], F32, tag="yfT")
                ybT = bp.tile([128, 32 * 256], F32, tag="ybT")
                att = bp.tile([DIM, 2 * 256], F32, tag="att")
                attT = bp.tile([DIM, 2 * 256], F32, tag="attT")
                img_bf = bp.tile([DIM, 2 * NJ * 256], BF16, tag="img")
                f1in = bp.tile([DIM, 2 * SLA], BF16, tag="f1in")
                f2in = bp.tile([DIM, 2 * GRP], BF16, tag="f2in")

                # ---- transposes of y_f, y_b ----
                with tc.tile_pool(name="tp", bufs=3) as tpp, \
                     tc.tile_pool(name="tps", bufs=2, space="PSUM") as tps:
                    for (src, dst) in ((y_f, yfT), (y_b, ybT)):
                        for lt in range(32):
                            tp = tps.tile([128, 256], F32, tag="tp")
                            for h in range(2):
                                chunk = tpp.tile([128, 128], F32, tag="ch")
                                nc.sync.dma_start(
                                    chunk[:],
                                    src[:, h * L + lt * 128:
                                        h * L + (lt + 1) * 128])
                                nc.tensor.transpose(
                                    tp[:, h * 128:(h + 1) * 128], chunk[:],
                                    id_t[:])
                            nc.scalar.copy(dst[:, lt * 256:(lt + 1) * 256],
                                           tp[:])

                # ---- G + softmax -> att [d, e], then attT ----
                with tc.tile_pool(name="smx", bufs=2) as wk, \
                     tc.tile_pool(name="gps", bufs=2, space="PSUM") as gpp:
                    for h in range(2):
                        gps = gpp.tile([128, 256], F32, tag="gps")
                        for lt in range(32):
                            nc.tensor.matmul(
                                gps[:],
                                yfT[:, lt * 256 + h * 128:
                                    lt * 256 + (h + 1) * 128],
                                ybT[:, lt * 256:(lt + 1) * 256],
                                start=(lt == 0), stop=(lt == 31))
                        mx = wk.tile([128, 1], F32, tag="mx")
                        nc.vector.tensor_reduce(mx[:], gps[:],
                                                mybir.AxisListType.X, OP.max)
                        nmx = wk.tile([128, 1], F32, tag="nmx")
                        nc.vector.tensor_scalar_mul(nmx[:], mx[:], -1.0)
                        ex = wk.tile([128, 256], F32, tag="ex")
                        sm = wk.tile([128, 1], F32, tag="sm")
                        nc.scalar.activation(ex[:], gps[:], AF.Exp, bias=nmx[:],
                                             accum_out=sm[:])
                        rs = wk.tile([128, 1], F32, tag="rs")
                        nc.vector.reciprocal(rs[:], sm[:])
                        nc.vector.tensor_scalar_mul(
                            att[:, h * 256:(h + 1) * 256], ex[:], rs[:])
                    for h in range(2):
                        for g in range(2):
                            tp2 = gpp.tile([128, 128], F32, tag="tp2")
                            nc.tensor.transpose(
                                tp2[:],
                                att[:, h * 256 + g * 128:
                                    h * 256 + (g + 1) * 128], id_t[:])
                            nc.scalar.copy(
                                attT[:, g * 256 + h * 128:
                                     g * 256 + (h + 1) * 128], tp2[:])

                # ---- out_a_img slab ----
                with tc.tile_pool(name="oa", bufs=1) as oap, \
                     tc.tile_pool(name="oaps", bufs=2, space="PSUM") as oaps:
                    ysl = oap.tile([DIM, 2 * NJ * 256], F32, tag="ysl")
                    nc.sync.dma_start(ysl[:], y_s_sl)
                    for j in range(NJ):
                        for m in range(2):
                            aps = oaps.tile([128, 256], F32, tag="aps")
                            for h in range(2):
                                nc.tensor.matmul(
                                    aps[:],
                                    ysl[:, h * NJ * 256 + j * 256 + m * 128:
                                        h * NJ * 256 + j * 256 + (m + 1) * 128],
                                    attT[:, h * 256:(h + 1) * 256],
                                    start=(h == 0), stop=(h == 1))
                            nc.scalar.copy(
                                img_bf[:, m * NJ * 256 + j * 256:
                                       m * NJ * 256 + (j + 1) * 256], aps[:])

                # ---- out_m window ----
                with tc.tile_pool(name="om", bufs=1) as om, \
                     tc.tile_pool(name="domp", bufs=1, space="DRAM") as domp, \
                     tc.tile_pool(name="omps", bufs=2, space="PSUM") as omps:
                    xw_t = om.tile([DIM, WIN], F32, tag="xw")
                    nc.sync.dma_start(xw_t[:], x_slab)
                    xn = _layernorm(nc, om, omps, domp, xw_t, lnw_t, lnb_t,
                                    wmean_t, WIN, "b")
                    sz = om.tile([DIM, 2 * WIN], F32, tag="sz")
                    for i in range(WIN // 256):
                        sl = slice(i * 256, (i + 1) * 256)
                        for h in range(2):
                            zps = omps.tile([128, 256], F32, tag="zps")
                            nc.tensor.matmul(
                                zps[:], wz_t[:, h * 128:(h + 1) * 128],
                                xn[:, sl])
                            nc.scalar.activation(
                                sz[:, h * WIN + i * 256:h * WIN + (i + 1) * 256],
                                zps[:], AF.Silu)
                    ysum = om.tile([DIM, 2 * WIN], F32, tag="ysum")
                    tmp = om.tile([DIM, 2 * WIN], F32, tag="tmpw")
                    nc.sync.dma_start(ysum[:], y_f_w)
                    nc.sync.dma_start(tmp[:], y_b_w)
                    nc.vector.tensor_add(ysum[:], ysum[:], tmp[:])
                    tmp2 = om.tile([DIM, 2 * WIN], F32, tag="tmpw")
                    nc.sync.dma_start(tmp2[:], y_s_w)
                    nc.vector.tensor_add(ysum[:], ysum[:], tmp2[:])
                    nc.vector.tensor_mul(ysum[:], ysum[:], sz[:])
                    ys4 = ysum
                    out_m = om.tile([DIM, WIN], F32, tag="outm")
                    for i in range(WIN // 256):
                        sl = slice(i * 256, (i + 1) * 256)
                        mps2 = omps.tile([128, 256], F32, tag="mps2")
                        for h in range(2):
                            nc.tensor.matmul(
                                mps2[:], op_t[:, h * 128:(h + 1) * 128],
                                ys4[:, h * WIN + i * 256:
                                    h * WIN + (i + 1) * 256],
                                start=(h == 0), stop=(h == 1))
                        nc.scalar.copy(out_m[:, sl], mps2[:])

                    # ---- build conv slabs ----
                    nc.vector.memset(f1in[:], 0)
                    for m in range(2):
                        nc.vector.tensor_copy(
                            f1in[:, m * SLA:(m + 1) * SLA]
                                .rearrange("p (r w) -> p r w", w=66)[:, :, 1:65],
                            img_bf[:, m * NJ * 256:(m + 1) * NJ * 256]
                                .rearrange("p (r w) -> p r w", w=64))
                    nc.vector.memset(f2in[:], 0)
                    nc.vector.tensor_copy(
                        f2in[:, GRP + 1:GRP + 1 + GR]
                            .rearrange("p (r w) -> p r w", w=66)[:, :, 1:65],
                        out_m[:, 0:18 * 64]
                            .rearrange("p (r w) -> p r w", w=64))

                # ---- fuse1 conv: slab rows [3,21) ----
                with tc.tile_pool(name="cv", bufs=2) as cpo, \
                     tc.tile_pool(name="cvps", bufs=2, space="PSUM") as cvps:
                    for cidx in range(3):
                        f1ps = cvps.tile([128, 396], F32, tag="f1ps")
                        base = (3 + cidx * 6) * 66
                        first = True
                        for dy in (-1, 0, 1):
                            for dx in (-1, 0, 1):
                                off = base + dy * 66 + dx
                                wcol = ((dy + 1) * 3 + (dx + 1)) * 128
                                for h in range(2):
                                    nc.tensor.matmul(
                                        f1ps[:],
                                        f1w_t[:, h * 9 * DIM + wcol:
                                              h * 9 * DIM + wcol + 128],
                                        f1in[:, h * SLA + off:
                                             h * SLA + off + 396],
                                        start=first,
                                        stop=(dy == 1 and dx == 1 and h == 1))
                                    first = False
                        nc.scalar.activation(
                            f2in[:, 1 + cidx * 396:1 + (cidx + 1) * 396],
                            f1ps[:], AF.Identity, bias=f1b_t[:])
                    nc.vector.tensor_mul(f2in[:, 1:1 + GR], f2in[:, 1:1 + GR],
                                         mask_t[:])
                    nc.vector.tensor_mul(f2in[:, GRP + 1:GRP + 1 + GR],
                                         f2in[:, GRP + 1:GRP + 1 + GR],
                                         mask_t[:])

                    # ---- fuse2 conv: grid rows [1,17) ----
                    o_sb = cpo.tile([DIM, 1024], F32, tag="osb")
                    for cidx in range(4):
                        f2ps = cvps.tile([128, 264], F32, tag="f2ps")
                        base = (1 + cidx * 4) * 66
                        first = True
                        for dy in (-1, 0, 1):
                            for dx in (-1, 0, 1):
                                off = base + dy * 66 + dx
                                wcol = ((dy + 1) * 3 + (dx + 1)) * 128
                                for h in range(2):
                                    nc.tensor.matmul(
                                        f2ps[:],
                                        f2w_t[:, h * 9 * DIM + wcol:
                                              h * 9 * DIM + wcol + 128],
                                        f2in[:, h * GRP + 1 + off:
                                             h * GRP + 1 + off + 264],
                                        start=first,
                                        stop=(dy == 1 and dx == 1 and h == 1))
                                    first = False
                        nc.scalar.activation(
                            o_sb[:, cidx * 256:(cidx + 1) * 256]
                                .rearrange("p (r w) -> p r w", w=64),
                            f2ps[:].rearrange("p (r w) -> p r w",
                                              w=66)[:, :, 1:65],
                            AF.Identity, bias=f2b_t[:])
                    xr = cpo.tile([DIM, 1024], F32, tag="xr")
                    nc.sync.dma_start(xr[:], x_res)
                    o2 = cpo.tile([DIM, 1024], F32, tag="o2")
                    nc.vector.tensor_add(o2[:], o_sb[:], xr[:])
                    nc.sync.dma_start(o_out, o2[:])
    _split_excess_waits(nc)
    return nc


# ---------------------------------------------------------------------------
# Host glue
# ---------------------------------------------------------------------------
_CACHE = {}


def _get_ncs():
    if "scan" not in _CACHE:
        _CACHE["scan"] = build_scan_nc()
        _CACHE["post"] = build_post_nc()
    return _CACHE["scan"], _CACHE["post"]


def _perm():
    return np.arange(L).reshape(NSLICES, L // NSLICES).T.reshape(-1)


def pack2(a):
    """[256, X] -> [128, 2X] half-major."""
    a = np.asarray(a, np.float32)
    return np.ascontiguousarray(np.concatenate([a[:128], a[128:]], axis=1))


def unpack2(a):
    """[128, 2X] -> [256, X]."""
    X = a.shape[1] // 2
    return np.ascontiguousarray(np.concatenate([a[:, :X], a[:, X:]], axis=0))


def _scan_inmaps(inputs):
    x = np.asarray(inputs["x"], np.float32)
    perm = _perm()
    com = {
        "w_u_T": np.ascontiguousarray(
            np.asarray(inputs["in_proj_w"], np.float32)[:D_INNER].T),
        "ln_w": np.asarray(inputs["ln_w"], np.float32).reshape(DIM, 1),
        "ln_b": np.asarray(inputs["ln_b"], np.float32).reshape(DIM, 1),
        "w_mean": np.full((DIM, 1), 1.0 / DIM, np.float32),
    }
    maps = []
    for br in ("f", "b", "s"):
        brm = {
            "conv_w": pack2(np.asarray(inputs[f"conv_w_{br}"],
                                       np.float32)[:, 0, :]),
            "conv_b": pack2(np.asarray(inputs[f"conv_b_{br}"],
                                       np.float32).reshape(D_INNER, 1)),
            "xproj_T": pack2(np.asarray(inputs[f"xproj_w_{br}"],
                                        np.float32).T),
            "dtw_T": np.ascontiguousarray(
                np.asarray(inputs[f"dtproj_w_{br}"], np.float32).T),
            "dtb": pack2(np.asarray(inputs[f"dtproj_b_{br}"],
                                    np.float32).reshape(D_INNER, 1)),
            "A_mat": pack2(-np.exp(np.asarray(inputs[f"A_log_{br}"],
                                              np.float32))),
            "Dvec": pack2(np.asarray(inputs[f"D_{br}"],
                                     np.float32).reshape(D_INNER, 1)),
        }
        for b in range(B_SZ):
            xl = x[b].reshape(DIM, L)
            if br == "b":
                xl = xl[:, ::-1]
            elif br == "s":
                xl = xl[:, perm]
            m = dict(com)
            m.update(brm)
            m["xs"] = np.ascontiguousarray(xl)
            maps.append(m)
    maps.append(dict(maps[0]))
    maps.append(dict(maps[0]))
    return maps


def _post_inmaps(inputs, y_f, y_b, y_s):
    x = np.asarray(inputs["x"], np.float32)
    wfull = np.asarray(inputs["in_proj_w"], np.float32)
    f1wp = np.zeros((D_INNER, 9 * DIM), np.float32)
    f2wp = np.zeros((D_INNER, 9 * DIM), np.float32)
    for dy in range(3):
        for dx in range(3):
            s = dy * 3 + dx
            f1wp[:, s * 128:(s + 1) * 128] = \
                np.asarray(inputs["fuse1_w"], np.float32)[:, :, dy, dx].T
            f2wp[:, s * 128:(s + 1) * 128] = \
                np.asarray(inputs["fuse2_w"], np.float32)[:, :, dy, dx].T
    com = {
        "w_z_T": np.ascontiguousarray(wfull[D_INNER:].T),
        "ln_w": np.asarray(inputs["ln_w"], np.float32).reshape(DIM, 1),
        "ln_b": np.asarray(inputs["ln_b"], np.float32).reshape(DIM, 1),
        "w_mean": np.full((DIM, 1), 1.0 / DIM, np.float32),
        "outp_T": pack2(np.asarray(inputs["out_proj_w"], np.float32).T),
        "f1w": pack2(f1wp).astype(ml_dtypes.bfloat16),
        "f1b": np.asarray(inputs["fuse1_b"], np.float32).reshape(DIM, 1),
        "f2w": pack2(f2wp).astype(ml_dtypes.bfloat16),
        "f2b": np.asarray(inputs["fuse2_b"], np.float32).reshape(DIM, 1),
        "ident": np.eye(128, dtype=np.float32),
    }
    maps = []
    for c in range(8):
        b, q = c // 4, c % 4
        m = dict(com)
        m["y_f"] = pack2(y_f[b])
        m["y_b"] = pack2(y_b[b])
        ysl = np.zeros((D_INNER, NJ * 256), np.float32)
        for ji in range(NJ):
            j0 = 4 * q - 1 + ji
            if 0 <= j0 < 16:
                ysl[:, ji * 256:(ji + 1) * 256] = y_s[b][:, j0::16]
        m["y_s_sl"] = pack2(ysl)
        lo = 64 * (16 * q - 1)
        idx = lo + np.arange(WIN)
        valid = (idx >= 0) & (idx < L)
        idxc = np.clip(idx, 0, L - 1)

        def win(a):
            w = a[:, idxc].copy()
            w[:, ~valid] = 0.0
            return w

        m["y_f_w"] = pack2(win(y_f[b]))
        m["y_b_w"] = pack2(win(y_b[b]))
        m["y_s_w"] = pack2(win(y_s[b]))
        m["x_slab"] = np.ascontiguousarray(win(x[b].reshape(DIM, L)))
        m["x_res"] = np.ascontiguousarray(
            x[b].reshape(DIM, L)[:, 1024 * q:1024 * (q + 1)])
        msk = np.zeros((18, 66), np.float32)
        for r in range(18):
            if 0 <= (16 * q - 1 + r) < 64:
                msk[r, 1:65] = 1.0
        m["mask"] = np.ascontiguousarray(
            np.broadcast_to(msk.reshape(1, GR), (DIM, GR)))
        maps.append(m)
    return maps


def run_host_glue(scan_results):
    perm = _perm()
    y_f, y_b, y_s = {}, {}, {}
    for b in range(B_SZ):
        y_f[b] = unpack2(scan_results[0 * 2 + b]["y_out"])
        y_b[b] = np.ascontiguousarray(
            unpack2(scan_results[1 * 2 + b]["y_out"])[:, ::-1])
        ysn = np.empty((D_INNER, L), np.float32)
        ysn[:, perm] = unpack2(scan_results[2 * 2 + b]["y_out"])
        y_s[b] = ysn
    return y_f, y_b, y_s


def kernel(**inputs):
    nc_scan, nc_post = _get_ncs()
    scan_maps = _scan_inmaps(inputs)
    res_a = bass_utils.run_bass_kernel_spmd(nc_scan, scan_maps,
                                            core_ids=list(range(8)))
    y_f, y_b, y_s = run_host_glue(res_a.results)
    post_maps = _post_inmaps(inputs, y_f, y_b, y_s)
    res_b = bass_utils.run_bass_kernel_spmd(nc_post, post_maps,
                                            core_ids=list(range(8)))
    out = np.empty((B_SZ, DIM, H_IMG, W_IMG), np.float32)
    for c in range(8):
        b, q = c // 4, c % 4
        out[b, :, 16 * q:16 * (q + 1), :] = \
            res_b.results[c]["o_out"].reshape(DIM, 16, 64)
    return out


# revision 15
# speedup vs baseline: 6979.7960x; 6979.7960x over previous
"""MFABlock Trainium2 kernel: 2-launch SPMD implementation.

d_inner=256 tensors are packed half-major: [128 partitions, 2*X free], where
half h of channel d (= h*128 + p) occupies free columns [h*X, (h+1)*X).

Launch A (6 of 8 cores): per-(branch, batch) full-L mamba scan; host
pre-reverses / pre-permutes x per branch so all cores run identical code.
Launch B (8 cores): channel attention + fuse convs; core (b, q) emits output
spatial rows [16q, 16q+16) of batch b.
"""
import sys
sys.path.insert(0, "/opt/trn_rl_repo")

import numpy as np
import ml_dtypes
import concourse.bass as bass
import concourse.mybir as mybir
import concourse.tile as tile
from concourse import bass_utils
from concourse.vector_clock import ScopedClock

F32 = mybir.dt.float32
BF16 = mybir.dt.bfloat16
AF = mybir.ActivationFunctionType
OP = mybir.AluOpType

DIM = 128
D_STATE = 16
D_CONV = 4
D_INNER = 256
DT_RANK = 8
NSLICES = 4
B_SZ, H_IMG, W_IMG = 2, 64, 64
L = H_IMG * W_IMG          # 4096
NCHUNK = 4
FD = L // NCHUNK           # 1024
CH = 512                   # pre-stage chunk
NP = DT_RANK + 2 * D_STATE  # 40

NJ = 6                     # j0 window (uniform)
WIN = 20 * 64              # out_m l-window (rows 16q-1 .. 16q+19)
GR = 18 * 66               # fuse2-in padded grid (per ic-half)
GRP = GR + 2               # +2 slack for the (+1,+1) shifted read
SLA = 24 * 66              # fuse1-in padded grid (per ic-half)
EPS = 1e-5


def _patch_tile_drain():
    """Container's walrus rejects >1 sem-wait on the SP drain at TileContext
    exit; split the global-clock waits onto standalone NOPs."""
    if getattr(tile.TileContext, "_drain_patched", False):
        return

    def _patched(self, tick_clock, wait_clock):
        nc = self.nc
        probe = nc.sync.nop(nofuse=True)
        wait_clock.add_sem_waits(
            probe.ins, ScopedClock({None: tick_clock.global_clock})
        )
        si = probe.ins.sync_info
        if si is not None and len(si.on_wait) > 1:
            waits = list(si.on_wait)
            si.on_wait = waits[:1]
            for w in waits[1:]:
                extra = nc.sync.nop(nofuse=True)
                extra.ins.sync_info = mybir.SyncInfo(on_wait=[w], on_update=[])
        nc.sync.drain()
        nc.all_engine_barrier()
        assert self.sems is not None
        popped = nc._tile_sem_poison_stack.pop()
        assert popped is self._sem_poison
        nc.clear_and_free_semaphores(list(self.sems.allocated().values()))
        nc.all_engine_barrier()

    tile.TileContext._drain_and_barrier = _patched
    tile.TileContext._drain_patched = True




_WSPLIT_CTR = [0]


def _split_excess_waits(nc, max_waits=1):
    """Walrus in this container rejects >1 sem-wait on many instruction
    structs; hoist excess waits onto same-engine NOPs placed just before."""
    for fn in nc.m.functions:
        for bb in fn.blocks:
            new_insts = []
            for inst in bb.instructions:
                si = inst.sync_info
                if si is not None and len(si.on_wait) > max_waits:
                    waits = list(si.on_wait)
                    for w in waits[:-max_waits]:
                        _WSPLIT_CTR[0] += 1
                        nop = mybir.InstNoOp(
                            name=f"I-wsplit-{_WSPLIT_CTR[0]}", ins=[], outs=[])
                        nop.engine = inst.engine
                        nop.sync_info = mybir.SyncInfo(on_wait=[w],
                                                       on_update=[])
                        new_insts.append(nop)
                        nc.register_instruction(nop, overwrite=True)
                    si.on_wait = waits[-max_waits:]
                new_insts.append(inst)
            bb.instructions = new_insts


def _layernorm(nc, pool, pps, dp, xw_t, lnw_t, lnb_t, wmean_t, width, tag):
    """LN over the 128 partitions of xw_t [128, width] -> xn tile."""
    sq = pool.tile([DIM, width], F32, tag=tag + "sq")
    nc.scalar.activation(sq[:], xw_t[:], AF.Square)
    stats = pool.tile([1, 2 * width], F32, tag=tag + "st")
    NMM = 256
    for i in range(width // NMM):
        sl = slice(i * NMM, (i + 1) * NMM)
        stp = pps.tile([1, 2 * NMM], F32, tag=tag + "stp")
        nc.tensor.matmul(stp[:, 0:NMM], wmean_t[:], xw_t[:, sl])
        nc.tensor.matmul(stp[:, NMM:2 * NMM], wmean_t[:], sq[:, sl])
        nc.scalar.copy(stats[:, i * NMM:(i + 1) * NMM], stp[:, 0:NMM])
        nc.scalar.copy(stats[:, width + i * NMM:width + (i + 1) * NMM],
                       stp[:, NMM:2 * NMM])
    musq = pool.tile([1, width], F32, tag=tag + "mq")
    nc.scalar.activation(musq[:], stats[:, 0:width], AF.Square)
    var = pool.tile([1, width], F32, tag=tag + "var")
    nc.vector.tensor_sub(var[:], stats[:, width:2 * width], musq[:])
    eps_t = pool.tile([1, 1], F32, tag=tag + "eps")
    nc.vector.memset(eps_t[:], EPS)
    lv = pool.tile([1, width], F32, tag=tag + "sd")
    nc.scalar.activation(lv[:], var[:], AF.Ln, bias=eps_t[:])
    rr = pool.tile([1, width], F32, tag=tag + "rr")
    nc.scalar.activation(rr[:], lv[:], AF.Exp, scale=-0.5)
    mr = pool.tile([1, width], F32, tag=tag + "mr")
    nc.vector.tensor_mul(mr[:], stats[:, 0:width], rr[:])
    rowd = dp.tile([2, width], F32, tag=tag + "rowd")
    nc.sync.dma_start(rowd[0:1, :], rr[:])
    nc.sync.dma_start(rowd[1:2, :], mr[:])
    R128 = pool.tile([DIM, width], F32, tag=tag + "R")
    nc.sync.dma_start(R128[:], rowd[0:1, :].partition_broadcast(DIM))
    M128 = pool.tile([DIM, width], F32, tag=tag + "M")
    nc.sync.dma_start(M128[:], rowd[1:2, :].partition_broadcast(DIM))
    t1 = pool.tile([DIM, width], F32, tag=tag + "t1")
    nc.vector.tensor_mul(t1[:], xw_t[:], R128[:])
    nc.vector.tensor_sub(t1[:], t1[:], M128[:])
    nc.vector.tensor_scalar(t1[:], t1[:], lnw_t[:], lnb_t[:], OP.mult, OP.add)
    return t1


# ---------------------------------------------------------------------------
# Launch A
# ---------------------------------------------------------------------------
def build_scan_nc():
    _patch_tile_drain()
    nc = bass.Bass("TRN2", num_devices=8, debug=False)
    xs = nc.dram_tensor("xs", [DIM, L], F32, kind="ExternalInput").ap()
    w_u_T = nc.dram_tensor("w_u_T", [DIM, D_INNER], F32, kind="ExternalInput").ap()
    ln_w = nc.dram_tensor("ln_w", [DIM, 1], F32, kind="ExternalInput").ap()
    ln_b = nc.dram_tensor("ln_b", [DIM, 1], F32, kind="ExternalInput").ap()
    w_mean = nc.dram_tensor("w_mean", [DIM, 1], F32, kind="ExternalInput").ap()
    conv_w = nc.dram_tensor("conv_w", [DIM, 2 * D_CONV], F32,
                            kind="ExternalInput").ap()
    conv_b = nc.dram_tensor("conv_b", [DIM, 2], F32, kind="ExternalInput").ap()
    xproj_T = nc.dram_tensor("xproj_T", [DIM, 2 * NP], F32,
                             kind="ExternalInput").ap()
    dtw_T = nc.dram_tensor("dtw_T", [DT_RANK, D_INNER], F32,
                           kind="ExternalInput").ap()
    dtb = nc.dram_tensor("dtb", [DIM, 2], F32, kind="ExternalInput").ap()
    A_mat = nc.dram_tensor("A_mat", [DIM, 2 * D_STATE], F32,
                           kind="ExternalInput").ap()
    Dvec = nc.dram_tensor("Dvec", [DIM, 2], F32, kind="ExternalInput").ap()
    y_out = nc.dram_tensor("y_out", [DIM, 2 * L], F32, kind="ExternalOutput").ap()

    LP = L + 3  # padded per-half width for conv input

    with tile.TileContext(nc) as tc:
        with tc.tile_pool(name="const", bufs=1) as cpool:
            lnw_t = cpool.tile([DIM, 1], F32); nc.sync.dma_start(lnw_t[:], ln_w)
            lnb_t = cpool.tile([DIM, 1], F32); nc.sync.dma_start(lnb_t[:], ln_b)
            wmean_t = cpool.tile([DIM, 1], F32)
            nc.sync.dma_start(wmean_t[:], w_mean)
            wu_t = cpool.tile([DIM, D_INNER], F32)
            nc.sync.dma_start(wu_t[:], w_u_T)
            cw_t = cpool.tile([DIM, 2 * D_CONV], F32)
            nc.sync.dma_start(cw_t[:], conv_w)
            cb_t = cpool.tile([DIM, 2], F32); nc.sync.dma_start(cb_t[:], conv_b)
            xp_t = cpool.tile([DIM, 2 * NP], F32)
            nc.sync.dma_start(xp_t[:], xproj_T)
            dtw_t = cpool.tile([DT_RANK, D_INNER], F32)
            nc.sync.dma_start(dtw_t[:], dtw_T)
            dtb_t = cpool.tile([DIM, 2], F32); nc.sync.dma_start(dtb_t[:], dtb)
            A_t = cpool.tile([DIM, 2 * D_STATE], F32)
            nc.sync.dma_start(A_t[:], A_mat)
            D_t = cpool.tile([DIM, 2], F32); nc.sync.dma_start(D_t[:], Dvec)

            with tc.tile_pool(name="persist", bufs=1) as pp:
                u_bf = pp.tile([DIM, 2 * LP], BF16, tag="ubf")
                uc_t = pp.tile([DIM, 2 * L], F32, tag="uc")
                delta_t = pp.tile([DIM, 2 * L], F32, tag="delta")
                du_t = pp.tile([DIM, 2 * L], BF16, tag="du")
                bc_t = pp.tile([NP, L], BF16, tag="bc")
                yacc_t = pp.tile([DIM, 2 * L], F32, tag="yacc")

                # ---------- pre: LN + in_proj (chunked) ----------
                for h in range(2):
                    nc.vector.memset(u_bf[:, h * LP:h * LP + 3], 0)
                with tc.tile_pool(name="pre", bufs=2) as prep, \
                     tc.tile_pool(name="lnp", bufs=1) as lnp, \
                     tc.tile_pool(name="dpre", bufs=2, space="DRAM") as dpre, \
                     tc.tile_pool(name="pps", bufs=2, space="PSUM") as pps:
                    for c in range(L // CH):
                        csl = slice(c * CH, (c + 1) * CH)
                        xc = prep.tile([DIM, CH], F32, tag="xc")
                        nc.sync.dma_start(xc[:], xs[:, csl])
                        xn = _layernorm(nc, lnp, pps, dpre, xc, lnw_t, lnb_t,
                                        wmean_t, CH, "a")
                        for i in range(CH // 512):
                            sl = slice(i * 512, (i + 1) * 512)
                            for h in range(2):
                                ups = pps.tile([128, 512], F32, tag="ups")
                                nc.tensor.matmul(
                                    ups[:], wu_t[:, h * 128:(h + 1) * 128],
                                    xn[:, sl])
                                g0 = h * LP + 3 + c * CH + i * 512
                                nc.scalar.copy(u_bf[:, g0:g0 + 512], ups[:])

                # ---------- conv + silu + projections ----------
                with tc.tile_pool(name="mid", bufs=1) as midp, \
                     tc.tile_pool(name="mps", bufs=2, space="PSUM") as mps:
                    accs = [None, None]
                    for h in range(2):
                        b0 = h * LP
                        a0 = midp.tile([DIM, L], BF16, tag=f"cva{h}")
                        nc.vector.tensor_scalar_mul(
                            a0[:], u_bf[:, b0:b0 + L], cw_t[:, h * 4:h * 4 + 1])
                        a1 = midp.tile([DIM, L], BF16, tag=f"cvb{h}")
                        nc.vector.scalar_tensor_tensor(
                            a1[:], u_bf[:, b0 + 1:b0 + 1 + L],
                            cw_t[:, h * 4 + 1:h * 4 + 2], a0[:], OP.mult, OP.add)
                        a2 = midp.tile([DIM, L], BF16, tag=f"cva{h}")
                        nc.vector.scalar_tensor_tensor(
                            a2[:], u_bf[:, b0 + 2:b0 + 2 + L],
                            cw_t[:, h * 4 + 2:h * 4 + 3], a1[:], OP.mult, OP.add)
                        a3 = midp.tile([DIM, L], BF16, tag=f"cvb{h}")
                        nc.vector.scalar_tensor_tensor(
                            a3[:], u_bf[:, b0 + 3:b0 + 3 + L],
                            cw_t[:, h * 4 + 3:h * 4 + 4], a2[:], OP.mult, OP.add)
                        nc.scalar.activation(uc_t[:, h * L:(h + 1) * L], a3[:],
                                             AF.Silu, bias=cb_t[:, h:h + 1])

                    xz = midp.tile([NP, L], F32, tag="xz")
                    for i in range(L // 512):
                        sl = slice(i * 512, (i + 1) * 512)
                        xps = mps.tile([NP, 512], F32, tag="xps")
                        for h in range(2):
                            nc.tensor.matmul(
                                xps[:], xp_t[:, h * NP:(h + 1) * NP],
                                uc_t[:, h * L + i * 512:h * L + (i + 1) * 512],
                                start=(h == 0), stop=(h == 1))
                        nc.scalar.copy(xz[:, sl], xps[:])
                    nc.vector.tensor_copy(bc_t[:], xz[:])
                    one_t = midp.tile([DIM, 1], F32, tag="one")
                    nc.vector.memset(one_t[:], 1.0)
                    for i in range(L // 512):
                        sl = slice(i * 512, (i + 1) * 512)
                        for h in range(2):
                            dps = mps.tile([128, 512], F32, tag="dps")
                            nc.tensor.matmul(
                                dps[:], dtw_t[:, h * 128:(h + 1) * 128],
                                xz[0:DT_RANK, sl])
                            edt = midp.tile([128, 512], F32, tag="edt")
                            nc.scalar.activation(edt[:], dps[:], AF.Exp,
                                                 bias=dtb_t[:, h:h + 1])
                            nc.scalar.activation(
                                delta_t[:, h * L + i * 512:
                                        h * L + (i + 1) * 512],
                                edt[:], AF.Ln, bias=one_t[:])
                    nc.vector.tensor_mul(du_t[:], delta_t[:], uc_t[:])

                # ---------- scan: n-loop ----------
                with tc.tile_pool(name="scan", bufs=2) as sp, \
                     tc.tile_pool(name="dsc", bufs=1, space="DRAM") as dsc, \
                     tc.tile_pool(name="scan3", bufs=3) as sp3:
                    bc_d = dsc.tile([NP, L], BF16, tag="bcd")
                    nc.sync.dma_start(bc_d[:], bc_t[:])
                    for n in range(D_STATE):
                        hprev = None
                        for c in range(NCHUNK):
                            sl = slice(c * FD, (c + 1) * FD)
                            Bb = sp.tile([DIM, FD], BF16, tag="Bb")
                            nc.sync.dma_start(
                                Bb[:],
                                bc_d[DT_RANK + n:DT_RANK + n + 1, sl].partition_broadcast(DIM))
                            Cb = sp.tile([DIM, FD], BF16, tag="Cb")
                            nc.sync.dma_start(
                                Cb[:],
                                bc_d[DT_RANK + D_STATE + n:DT_RANK + D_STATE + n + 1,
                                     sl].partition_broadcast(DIM))
                            dA = sp.tile([DIM, 2 * FD], BF16, tag="dA")
                            dBu = sp.tile([DIM, 2 * FD], BF16, tag="dBu")
                            hsc = sp3.tile([DIM, 2 * FD], BF16, tag="h")
                            hC = sp.tile([DIM, 2 * FD], BF16, tag="hC")
                            for h in range(2):
                                hf = slice(h * FD, (h + 1) * FD)
                                dsl = slice(h * L + c * FD, h * L + (c + 1) * FD)
                                nc.scalar.activation(
                                    dA[:, hf], delta_t[:, dsl], AF.Exp,
                                    scale=A_t[:, h * D_STATE + n:
                                              h * D_STATE + n + 1])
                                nc.gpsimd.tensor_mul(dBu[:, hf], du_t[:, dsl],
                                                     Bb[:])
                                init = (0.0 if c == 0 else
                                        hprev[:, h * FD + FD - 1:h * FD + FD])
                                nc.vector.tensor_tensor_scan(
                                    hsc[:, hf], dA[:, hf], dBu[:, hf], init,
                                    OP.mult, OP.add)
                                nc.vector.tensor_mul(hC[:, hf], hsc[:, hf],
                                                     Cb[:])
                                ysl = slice(h * L + c * FD,
                                            h * L + (c + 1) * FD)
                                if n == 0:
                                    nc.gpsimd.tensor_copy(yacc_t[:, ysl],
                                                          hC[:, hf])
                                else:
                                    nc.gpsimd.tensor_add(yacc_t[:, ysl],
                                                         yacc_t[:, ysl],
                                                         hC[:, hf])
                            hprev = hsc

                with tc.tile_pool(name="fin", bufs=1) as fp:
                    yfin = fp.tile([DIM, 2 * L], F32, tag="yfin")
                    for h in range(2):
                        hf = slice(h * L, (h + 1) * L)
                        nc.vector.scalar_tensor_tensor(
                            yfin[:, hf], uc_t[:, hf], D_t[:, h:h + 1],
                            yacc_t[:, hf], OP.mult, OP.add)
                    nc.sync.dma_start(y_out, yfin[:])
    _split_excess_waits(nc)
    return nc


# ---------------------------------------------------------------------------
# Launch B
# ---------------------------------------------------------------------------
def build_post_nc():
    _patch_tile_drain()
    nc = bass.Bass("TRN2", num_devices=8, debug=False)
    y_fT_d = nc.dram_tensor("y_fT", [128, 32 * 256], F32,
                            kind="ExternalInput").ap()
    y_bT_d = nc.dram_tensor("y_bT", [128, 32 * 256], F32,
                            kind="ExternalInput").ap()
    y_s_sl = nc.dram_tensor("y_s_sl", [DIM, 2 * NJ * 256], F32,
                            kind="ExternalInput").ap()
    y_f_w = nc.dram_tensor("y_f_w", [DIM, 2 * WIN], F32,
                           kind="ExternalInput").ap()
    y_b_w = nc.dram_tensor("y_b_w", [DIM, 2 * WIN], F32,
                           kind="ExternalInput").ap()
    y_s_w = nc.dram_tensor("y_s_w", [DIM, 2 * WIN], F32,
                           kind="ExternalInput").ap()
    x_slab = nc.dram_tensor("x_slab", [DIM, WIN], F32, kind="ExternalInput").ap()
    x_res = nc.dram_tensor("x_res", [DIM, 1024], F32, kind="ExternalInput").ap()
    w_z_T = nc.dram_tensor("w_z_T", [DIM, D_INNER], F32, kind="ExternalInput").ap()
    ln_w = nc.dram_tensor("ln_w", [DIM, 1], F32, kind="ExternalInput").ap()
    ln_b = nc.dram_tensor("ln_b", [DIM, 1], F32, kind="ExternalInput").ap()
    w_mean = nc.dram_tensor("w_mean", [DIM, 1], F32, kind="ExternalInput").ap()
    outp_T = nc.dram_tensor("outp_T", [DIM, 2 * DIM], F32,
                            kind="ExternalInput").ap()
    f1w = nc.dram_tensor("f1w", [DIM, 2 * 9 * DIM], BF16,
                         kind="ExternalInput").ap()
    f1b = nc.dram_tensor("f1b", [DIM, 1], F32, kind="ExternalInput").ap()
    f2w = nc.dram_tensor("f2w", [DIM, 2 * 9 * DIM], BF16,
                         kind="ExternalInput").ap()
    f2b = nc.dram_tensor("f2b", [DIM, 1], F32, kind="ExternalInput").ap()
    ident = nc.dram_tensor("ident", [128, 128], F32, kind="ExternalInput").ap()
    mask = nc.dram_tensor("mask", [DIM, GR], F32, kind="ExternalInput").ap()
    o_out = nc.dram_tensor("o_out", [DIM, 1024], F32, kind="ExternalOutput").ap()

    with tile.TileContext(nc) as tc:
        with tc.tile_pool(name="const", bufs=1) as cp:
            id_t = cp.tile([128, 128], F32); nc.sync.dma_start(id_t[:], ident)
            lnw_t = cp.tile([DIM, 1], F32); nc.sync.dma_start(lnw_t[:], ln_w)
            lnb_t = cp.tile([DIM, 1], F32); nc.sync.dma_start(lnb_t[:], ln_b)
            wmean_t = cp.tile([DIM, 1], F32); nc.sync.dma_start(wmean_t[:], w_mean)
            wz_t = cp.tile([DIM, D_INNER], F32); nc.sync.dma_start(wz_t[:], w_z_T)
            op_t = cp.tile([DIM, 2 * DIM], F32); nc.sync.dma_start(op_t[:], outp_T)
            f1w_t = cp.tile([DIM, 2 * 9 * DIM], BF16)
            nc.sync.dma_start(f1w_t[:], f1w)
            f1b_t = cp.tile([DIM, 1], F32); nc.sync.dma_start(f1b_t[:], f1b)
            f2w_t = cp.tile([DIM, 2 * 9 * DIM], BF16)
            nc.sync.dma_start(f2w_t[:], f2w)
            f2b_t = cp.tile([DIM, 1], F32); nc.sync.dma_start(f2b_t[:], f2b)
            mask_t = cp.tile([DIM, GR], F32); nc.sync.dma_start(mask_t[:], mask)

            with tc.tile_pool(name="big", bufs=1) as bp:
                yfT = bp.tile([128, 32 * 256], F32, tag="yfT")
                ybT = bp.tile([128, 32 * 256], F32, tag="ybT")
                att = bp.tile([DIM, 2 * 256], F32, tag="att")
                attT = bp.tile([DIM, 2 * 256], F32, tag="attT")
                img_bf = bp.tile([DIM, 2 * NJ * 256], BF16, tag="img")
                f1in = bp.tile([DIM, 2 * SLA], BF16, tag="f1in")
                f2in = bp.tile([DIM, 2 * GRP], BF16, tag="f2in")

                nc.sync.dma_start(yfT[:], y_fT_d)
                nc.sync.dma_start(ybT[:], y_bT_d)

                # ---- G + softmax -> att [d, e], then attT ----
                with tc.tile_pool(name="smx", bufs=2) as wk, \
                     tc.tile_pool(name="gps", bufs=2, space="PSUM") as gpp:
                    for h in range(2):
                        gps = gpp.tile([128, 256], F32, tag="gps")
                        for lt in range(32):
                            nc.tensor.matmul(
                                gps[:],
                                yfT[:, lt * 256 + h * 128:
                                    lt * 256 + (h + 1) * 128],
                                ybT[:, lt * 256:(lt + 1) * 256],
                                start=(lt == 0), stop=(lt == 31))
                        mx = wk.tile([128, 1], F32, tag="mx")
                        nc.vector.tensor_reduce(mx[:], gps[:],
                                                mybir.AxisListType.X, OP.max)
                        nmx = wk.tile([128, 1], F32, tag="nmx")
                        nc.vector.tensor_scalar_mul(nmx[:], mx[:], -1.0)
                        ex = wk.tile([128, 256], F32, tag="ex")
                        sm = wk.tile([128, 1], F32, tag="sm")
                        nc.scalar.activation(ex[:], gps[:], AF.Exp, bias=nmx[:],
                                             accum_out=sm[:])
                        rs = wk.tile([128, 1], F32, tag="rs")
                        nc.vector.reciprocal(rs[:], sm[:])
                        nc.vector.tensor_scalar_mul(
                            att[:, h * 256:(h + 1) * 256], ex[:], rs[:])
                    for h in range(2):
                        for g in range(2):
                            tp2 = gpp.tile([128, 128], F32, tag="tp2")
                            nc.tensor.transpose(
                                tp2[:],
                                att[:, h * 256 + g * 128:
                                    h * 256 + (g + 1) * 128], id_t[:])
                            nc.scalar.copy(
                                attT[:, g * 256 + h * 128:
                                     g * 256 + (h + 1) * 128], tp2[:])

                # ---- out_a_img slab ----
                with tc.tile_pool(name="oa", bufs=1) as oap, \
                     tc.tile_pool(name="oaps", bufs=2, space="PSUM") as oaps:
                    ysl = oap.tile([DIM, 2 * NJ * 256], F32, tag="ysl")
                    nc.sync.dma_start(ysl[:], y_s_sl)
                    for j in range(NJ):
                        for m in range(2):
                            aps = oaps.tile([128, 256], F32, tag="aps")
                            for h in range(2):
                                nc.tensor.matmul(
                                    aps[:],
                                    ysl[:, h * NJ * 256 + j * 256 + m * 128:
                                        h * NJ * 256 + j * 256 + (m + 1) * 128],
                                    attT[:, h * 256:(h + 1) * 256],
                                    start=(h == 0), stop=(h == 1))
                            nc.scalar.copy(
                                img_bf[:, m * NJ * 256 + j * 256:
                                       m * NJ * 256 + (j + 1) * 256], aps[:])

                # ---- out_m window ----
                with tc.tile_pool(name="om", bufs=1) as om, \
                     tc.tile_pool(name="domp", bufs=1, space="DRAM") as domp, \
                     tc.tile_pool(name="omps", bufs=2, space="PSUM") as omps:
                    xw_t = om.tile([DIM, WIN], F32, tag="xw")
                    nc.sync.dma_start(xw_t[:], x_slab)
                    xn = _layernorm(nc, om, omps, domp, xw_t, lnw_t, lnb_t,
                                    wmean_t, WIN, "b")
                    sz = om.tile([DIM, 2 * WIN], F32, tag="sz")
                    for i in range(WIN // 256):
                        sl = slice(i * 256, (i + 1) * 256)
                        for h in range(2):
                            zps = omps.tile([128, 256], F32, tag="zps")
                            nc.tensor.matmul(
                                zps[:], wz_t[:, h * 128:(h + 1) * 128],
                                xn[:, sl])
                            nc.scalar.activation(
                                sz[:, h * WIN + i * 256:h * WIN + (i + 1) * 256],
                                zps[:], AF.Silu)
                    ysum = om.tile([DIM, 2 * WIN], F32, tag="ysum")
                    tmp = om.tile([DIM, 2 * WIN], F32, tag="tmpw")
                    nc.sync.dma_start(ysum[:], y_f_w)
                    nc.sync.dma_start(tmp[:], y_b_w)
                    nc.vector.tensor_add(ysum[:], ysum[:], tmp[:])
                    tmp2 = om.tile([DIM, 2 * WIN], F32, tag="tmpw")
                    nc.sync.dma_start(tmp2[:], y_s_w)
                    nc.vector.tensor_add(ysum[:], ysum[:], tmp2[:])
                    nc.vector.tensor_mul(ysum[:], ysum[:], sz[:])
                    ys4 = ysum
                    out_m = om.tile([DIM, WIN], F32, tag="outm")
                    for i in range(WIN // 256):
                        sl = slice(i * 256, (i + 1) * 256)
                        mps2 = omps.tile([128, 256], F32, tag="mps2")
                        for h in range(2):
                            nc.tensor.matmul(
                                mps2[:], op_t[:, h * 128:(h + 1) * 128],
                                ys4[:, h * WIN + i * 256:
                                    h * WIN + (i + 1) * 256],
                                start=(h == 0), stop=(h == 1))
                        nc.scalar.copy(out_m[:, sl], mps2[:])

                    # ---- build conv slabs ----
                    nc.vector.memset(f1in[:], 0)
                    for m in range(2):
                        nc.vector.tensor_copy(
                            f1in[:, m * SLA:(m + 1) * SLA]
                                .rearrange("p (r w) -> p r w", w=66)[:, :, 1:65],
                            img_bf[:, m * NJ * 256:(m + 1) * NJ * 256]
                                .rearrange("p (r w) -> p r w", w=64))
                    nc.vector.memset(f2in[:], 0)
                    nc.vector.tensor_copy(
                        f2in[:, GRP + 1:GRP + 1 + GR]
                            .rearrange("p (r w) -> p r w", w=66)[:, :, 1:65],
                        out_m[:, 0:18 * 64]
                            .rearrange("p (r w) -> p r w", w=64))

                # ---- fuse1 conv: slab rows [3,21) ----
                with tc.tile_pool(name="cv", bufs=2) as cpo, \
                     tc.tile_pool(name="cvps", bufs=2, space="PSUM") as cvps:
                    for cidx in range(3):
                        f1ps = cvps.tile([128, 396], F32, tag="f1ps")
                        base = (3 + cidx * 6) * 66
                        first = True
                        for dy in (-1, 0, 1):
                            for dx in (-1, 0, 1):
                                off = base + dy * 66 + dx
                                wcol = ((dy + 1) * 3 + (dx + 1)) * 128
                                for h in range(2):
                                    nc.tensor.matmul(
                                        f1ps[:],
                                        f1w_t[:, h * 9 * DIM + wcol:
                                              h * 9 * DIM + wcol + 128],
                                        f1in[:, h * SLA + off:
                                             h * SLA + off + 396],
                                        start=first,
                                        stop=(dy == 1 and dx == 1 and h == 1))
                                    first = False
                        nc.scalar.activation(
                            f2in[:, 1 + cidx * 396:1 + (cidx + 1) * 396],
                            f1ps[:], AF.Identity, bias=f1b_t[:])
                    nc.vector.tensor_mul(f2in[:, 1:1 + GR], f2in[:, 1:1 + GR],
                                         mask_t[:])
                    nc.vector.tensor_mul(f2in[:, GRP + 1:GRP + 1 + GR],
                                         f2in[:, GRP + 1:GRP + 1 + GR],
                                         mask_t[:])

                    # ---- fuse2 conv: grid rows [1,17) ----
                    o_sb = cpo.tile([DIM, 1024], F32, tag="osb")
                    for cidx in range(4):
                        f2ps = cvps.tile([128, 264], F32, tag="f2ps")
                        base = (1 + cidx * 4) * 66
                        first = True
                        for dy in (-1, 0, 1):
                            for dx in (-1, 0, 1):
                                off = base + dy * 66 + dx
                                wcol = ((dy + 1) * 3 + (dx + 1)) * 128
                                for h in range(2):
                                    nc.tensor.matmul(
                                        f2ps[:],
                                        f2w_t[:, h * 9 * DIM + wcol:
                                              h * 9 * DIM + wcol + 128],
                                        f2in[:, h * GRP + 1 + off:
                                             h * GRP + 1 + off + 264],
                                        start=first,
                                        stop=(dy == 1 and dx == 1 and h == 1))
                                    first = False
                        nc.scalar.activation(
                            o_sb[:, cidx * 256:(cidx + 1) * 256]
                                .rearrange("p (r w) -> p r w", w=64),
                            f2ps[:].rearrange("p (r w) -> p r w",
                                              w=66)[:, :, 1:65],
                            AF.Identity, bias=f2b_t[:])
                    xr = cpo.tile([DIM, 1024], F32, tag="xr")
                    nc.sync.dma_start(xr[:], x_res)
                    o2 = cpo.tile([DIM, 1024], F32, tag="o2")
                    nc.vector.tensor_add(o2[:], o_sb[:], xr[:])
                    nc.sync.dma_start(o_out, o2[:])
    _split_excess_waits(nc)
    return nc


# ---------------------------------------------------------------------------
# Host glue
# ---------------------------------------------------------------------------
_CACHE = {}


def _get_ncs():
    if "scan" not in _CACHE:
        _CACHE["scan"] = build_scan_nc()
        _CACHE["post"] = build_post_nc()
    return _CACHE["scan"], _CACHE["post"]


def _perm():
    return np.arange(L).reshape(NSLICES, L // NSLICES).T.reshape(-1)


def pack2(a):
    """[256, X] -> [128, 2X] half-major."""
    a = np.asarray(a, np.float32)
    return np.ascontiguousarray(np.concatenate([a[:128], a[128:]], axis=1))


def unpack2(a):
    """[128, 2X] -> [256, X]."""
    X = a.shape[1] // 2
    return np.ascontiguousarray(np.concatenate([a[:, :X], a[:, X:]], axis=0))


def _scan_inmaps(inputs):
    x = np.asarray(inputs["x"], np.float32)
    perm = _perm()
    com = {
        "w_u_T": np.ascontiguousarray(
            np.asarray(inputs["in_proj_w"], np.float32)[:D_INNER].T),
        "ln_w": np.asarray(inputs["ln_w"], np.float32).reshape(DIM, 1),
        "ln_b": np.asarray(inputs["ln_b"], np.float32).reshape(DIM, 1),
        "w_mean": np.full((DIM, 1), 1.0 / DIM, np.float32),
    }
    maps = []
    for br in ("f", "b", "s"):
        brm = {
            "conv_w": pack2(np.asarray(inputs[f"conv_w_{br}"],
                                       np.float32)[:, 0, :]),
            "conv_b": pack2(np.asarray(inputs[f"conv_b_{br}"],
                                       np.float32).reshape(D_INNER, 1)),
            "xproj_T": pack2(np.asarray(inputs[f"xproj_w_{br}"],
                                        np.float32).T),
            "dtw_T": np.ascontiguousarray(
                np.asarray(inputs[f"dtproj_w_{br}"], np.float32).T),
            "dtb": pack2(np.asarray(inputs[f"dtproj_b_{br}"],
                                    np.float32).reshape(D_INNER, 1)),
            "A_mat": pack2(-np.exp(np.asarray(inputs[f"A_log_{br}"],
                                              np.float32))),
            "Dvec": pack2(np.asarray(inputs[f"D_{br}"],
                                     np.float32).reshape(D_INNER, 1)),
        }
        for b in range(B_SZ):
            xl = x[b].reshape(DIM, L)
            if br == "b":
                xl = xl[:, ::-1]
            elif br == "s":
                xl = xl[:, perm]
            m = dict(com)
            m.update(brm)
            m["xs"] = np.ascontiguousarray(xl)
            maps.append(m)
    maps.append(dict(maps[0]))
    maps.append(dict(maps[0]))
    return maps


def _post_inmaps(inputs, y_f, y_b, y_s):
    x = np.asarray(inputs["x"], np.float32)
    wfull = np.asarray(inputs["in_proj_w"], np.float32)
    f1wp = np.zeros((D_INNER, 9 * DIM), np.float32)
    f2wp = np.zeros((D_INNER, 9 * DIM), np.float32)
    for dy in range(3):
        for dx in range(3):
            s = dy * 3 + dx
            f1wp[:, s * 128:(s + 1) * 128] = \
                np.asarray(inputs["fuse1_w"], np.float32)[:, :, dy, dx].T
            f2wp[:, s * 128:(s + 1) * 128] = \
                np.asarray(inputs["fuse2_w"], np.float32)[:, :, dy, dx].T
    com = {
        "w_z_T": np.ascontiguousarray(wfull[D_INNER:].T),
        "ln_w": np.asarray(inputs["ln_w"], np.float32).reshape(DIM, 1),
        "ln_b": np.asarray(inputs["ln_b"], np.float32).reshape(DIM, 1),
        "w_mean": np.full((DIM, 1), 1.0 / DIM, np.float32),
        "outp_T": pack2(np.asarray(inputs["out_proj_w"], np.float32).T),
        "f1w": pack2(f1wp).astype(ml_dtypes.bfloat16),
        "f1b": np.asarray(inputs["fuse1_b"], np.float32).reshape(DIM, 1),
        "f2w": pack2(f2wp).astype(ml_dtypes.bfloat16),
        "f2b": np.asarray(inputs["fuse2_b"], np.float32).reshape(DIM, 1),
        "ident": np.eye(128, dtype=np.float32),
    }
    maps = []
    for c in range(8):
        b, q = c // 4, c % 4
        m = dict(com)
        # [l-tile-major, d-minor] layout: [128 l-part, 32*256]
        yft = y_f[b].T.reshape(32, 128, 256).transpose(1, 0, 2).reshape(
            128, 32 * 256)
        ybt = y_b[b].T.reshape(32, 128, 256).transpose(1, 0, 2).reshape(
            128, 32 * 256)
        m["y_fT"] = np.ascontiguousarray(yft)
        m["y_bT"] = np.ascontiguousarray(ybt)
        ysl = np.zeros((D_INNER, NJ * 256), np.float32)
        for ji in range(NJ):
            j0 = 4 * q - 1 + ji
            if 0 <= j0 < 16:
                ysl[:, ji * 256:(ji + 1) * 256] = y_s[b][:, j0::16]
        m["y_s_sl"] = pack2(ysl)
        lo = 64 * (16 * q - 1)
        idx = lo + np.arange(WIN)
        valid = (idx >= 0) & (idx < L)
        idxc = np.clip(idx, 0, L - 1)

        def win(a):
            w = a[:, idxc].copy()
            w[:, ~valid] = 0.0
            return w

        m["y_f_w"] = pack2(win(y_f[b]))
        m["y_b_w"] = pack2(win(y_b[b]))
        m["y_s_w"] = pack2(win(y_s[b]))
        m["x_slab"] = np.ascontiguousarray(win(x[b].reshape(DIM, L)))
        m["x_res"] = np.ascontiguousarray(
            x[b].reshape(DIM, L)[:, 1024 * q:1024 * (q + 1)])
        msk = np.zeros((18, 66), np.float32)
        for r in range(18):
            if 0 <= (16 * q - 1 + r) < 64:
                msk[r, 1:65] = 1.0
        m["mask"] = np.ascontiguousarray(
            np.broadcast_to(msk.reshape(1, GR), (DIM, GR)))
        maps.append(m)
    return maps


def run_host_glue(scan_results):
    perm = _perm()
    y_f, y_b, y_s = {}, {}, {}
    for b in range(B_SZ):
        y_f[b] = unpack2(scan_results[0 * 2 + b]["y_out"])
        y_b[b] = np.ascontiguousarray(
            unpack2(scan_results[1 * 2 + b]["y_out"])[:, ::-1])
        ysn = np.empty((D_INNER, L), np.float32)
        ysn[:, perm] = unpack2(scan_results[2 * 2 + b]["y_out"])
        y_s[b] = ysn
    return y_f, y_b, y_s


def kernel(**inputs):
    nc_scan, nc_post = _get_ncs()
    scan_maps = _scan_inmaps(inputs)
    res_a = bass_utils.run_bass_kernel_spmd(nc_scan, scan_maps,
                                            core_ids=list(range(8)))
    y_f, y_b, y_s = run_host_glue(res_a.results)
    post_maps = _post_inmaps(inputs, y_f, y_b, y_s)
    res_b = bass_utils.run_bass_kernel_spmd(nc_post, post_maps,
                                            core_ids=list(range(8)))
    out = np.empty((B_SZ, DIM, H_IMG, W_IMG), np.float32)
    for c in range(8):
        b, q = c // 4, c % 4
        out[b, :, 16 * q:16 * (q + 1), :] = \
            res_b.results[c]["o_out"].reshape(DIM, 16, 64)
    return out
